# revision 3
# baseline (speedup 1.0000x reference)
"""MoE (soft gumbel top-2 gate, hard top-1 forward) for 8 trn2 NeuronCores.

Forward math: with TOPK=2, the reference's straight-through ``hard`` weights
are exactly one_hot(argmax) in the forward pass, so
    out[t] = relu(x[t] @ Wfc[e*])**2 @ Wproj[e*],  e* = argmax_e(logits + g)[t]
plus the prior_logits output. Strategy:
  phase 1 (data-parallel): each core computes logits^T and the gumbel-argmax
          for its 1/8 of the tokens, on device.
  host:   buckets token indices by expert (pure indexing / layout, no math),
          gathers token rows per expert, pads to a common capacity C.
  phase 2 (expert-parallel): core e runs its expert's MLP over its tokens with
          float32r matmuls (full-rate, ~1e-4 rel err).
  host:   scatters rows back and assembles the two outputs.
"""
import numpy as np

import concourse.bacc as bacc
import concourse.tile as tile
from concourse import mybir
from concourse.bass_utils import run_bass_kernel_spmd
from concourse.masks import make_identity

P = 128
B, L, H, E = 4, 2048, 1024, 8
T = B * L                 # 8192 tokens
FF = 4 * H                # 4096
TPC = T // 8              # tokens per core in phase 1
KC1 = H // P              # 8  k-chunks over H
KC2 = FF // P             # 32 k-chunks over FF
NT = TPC // P             # 8  token tiles per core in phase 1

F32 = mybir.dt.float32
F32R = mybir.dt.float32r
U32 = mybir.dt.uint32
AF = mybir.ActivationFunctionType

_LAST_NC1 = None
_LAST_NC2 = None


def _build_phase1():
    nc = bacc.Bacc("TRN2", target_bir_lowering=False, debug=False)
    xTs_d = nc.dram_tensor("xTs", [P, KC1, TPC], F32, kind="ExternalInput").ap()
    wg_d = nc.dram_tensor("wg", [P, KC1, E], F32, kind="ExternalInput").ap()
    u_d = nc.dram_tensor("u_r", [P, NT, E], F32, kind="ExternalInput").ap()
    logT_d = nc.dram_tensor("logitsT", [E, TPC], F32, kind="ExternalOutput").ap()
    top1_d = nc.dram_tensor("top1", [P, NT], U32, kind="ExternalOutput").ap()

    with tile.TileContext(nc) as tc:
        with tc.tile_pool(name="sb", bufs=1) as sb, \
             tc.tile_pool(name="w2", bufs=2) as w2, \
             tc.tile_pool(name="psL", bufs=2, space="PSUM") as psLp, \
             tc.tile_pool(name="psT", bufs=2, space="PSUM") as psTp:
            xt = sb.tile([P, KC1, TPC], F32)
            nc.sync.dma_start(out=xt[:], in_=xTs_d[:])
            wg = sb.tile([P, KC1, E], F32)
            nc.sync.dma_start(out=wg[:], in_=wg_d[:])
            us = sb.tile([P, NT, E], F32)
            nc.sync.dma_start(out=us[:], in_=u_d[:])
            ident = sb.tile([E, E], F32)
            make_identity(nc, ident[:])

            # g' = ln(-ln(clip(u)));  z = logits - g'  (tau rescale is
            # argmax-invariant, softmax/top2-renorm don't affect the forward)
            uc = sb.tile([P, NT, E], F32)
            nc.vector.tensor_scalar(uc[:], us[:], 1e-9, scalar2=None,
                                    op0=mybir.AluOpType.max)
            nc.vector.tensor_scalar(uc[:], uc[:], 1.0 - 1e-9, scalar2=None,
                                    op0=mybir.AluOpType.min)
            t1 = sb.tile([P, NT, E], F32)
            nc.scalar.activation(t1[:], uc[:], AF.Ln)
            t2 = sb.tile([P, NT, E], F32)
            nc.scalar.activation(t2[:], t1[:], AF.Ln, scale=-1.0)

            # logits^T = Wg^T @ x^T, plain fp32 for argmax fidelity
            sbL = sb.tile([E, TPC], F32)
            for st in range(TPC // 512):
                psL = psLp.tile([E, 512], F32, tag="L")
                for kc in range(KC1):
                    nc.tensor.matmul(psL[:], lhsT=wg[:, kc, :],
                                     rhs=xt[:, kc, st * 512:(st + 1) * 512],
                                     start=(kc == 0), stop=(kc == KC1 - 1))
                nc.scalar.copy(sbL[:, st * 512:(st + 1) * 512], psL[:])
            nc.sync.dma_start(out=logT_d[:], in_=sbL[:])

            top1 = sb.tile([P, NT], U32)
            for tt in range(NT):
                psT = psTp.tile([P, E], F32, tag="T")
                nc.tensor.transpose(psT[:], in_=sbL[:, tt * P:(tt + 1) * P],
                                    identity=ident[:])
                z = w2.tile([P, E], F32, tag="z")
                nc.vector.tensor_tensor(out=z[:], in0=psT[:], in1=t2[:, tt, :],
                                        op=mybir.AluOpType.subtract)
                mx = w2.tile([P, E], F32, tag="mx")
                nc.vector.max(out=mx[:], in_=z[:])
                mi = w2.tile([P, E], U32, tag="mi")
                nc.vector.max_index(mi[:], mx[:], z[:])
                nc.vector.tensor_copy(top1[:, tt:tt + 1], mi[:, 0:1])
            nc.sync.dma_start(out=top1_d[:], in_=top1[:])
    nc.compile()
    return nc


def _build_phase2(C):
    """Expert MLP over C (padded) tokens: out = relu(Wfc^T x^T)^2^T Wproj."""
    nc = bacc.Bacc("TRN2", target_bir_lowering=False, debug=False)
    xT_d = nc.dram_tensor("xT", [P, KC1, C], F32, kind="ExternalInput").ap()
    wfc_d = nc.dram_tensor("wfc", [KC2, P, KC1, P], F32, kind="ExternalInput").ap()
    wpj_d = nc.dram_tensor("wpj", [2, KC2, P, 512], F32, kind="ExternalInput").ap()
    out_d = nc.dram_tensor("out", [C, H], F32, kind="ExternalOutput").ap()

    chunks = []
    t0 = 0
    while t0 < C:
        n = min(512, C - t0)
        chunks.append((t0, n))
        t0 += n

    with tile.TileContext(nc) as tc:
        with tc.tile_pool(name="xt", bufs=1) as xtp, \
             tc.tile_pool(name="hT", bufs=1) as hTp, \
             tc.tile_pool(name="wfc", bufs=3) as wfcp, \
             tc.tile_pool(name="wpj", bufs=6) as wpjp, \
             tc.tile_pool(name="tmp", bufs=3) as tmpp, \
             tc.tile_pool(name="osb", bufs=1) as osbp, \
             tc.tile_pool(name="psA", bufs=2, space="PSUM") as psAp, \
             tc.tile_pool(name="psB", bufs=1, space="PSUM") as psBp:
            xt = xtp.tile([P, KC1, C], F32R)
            nc.gpsimd.dma_start(out=xt[:], in_=xT_d[:])

            for (t0, N) in chunks:
                TC = N // P
                hT = hTp.tile([P, KC2, N], F32R, tag="hT")
                # ---- A: hiddenT = relu(Wfc^T @ xT_chunk)^2
                for fc in range(KC2):
                    pw = wfcp.tile([P, KC1, P], F32R, tag="wfc")
                    nc.gpsimd.dma_start(out=pw[:], in_=wfc_d[fc])
                    psA = psAp.tile([P, 512], F32, tag="A")
                    for kc in range(KC1):
                        nc.tensor.matmul(psA[:, :N], lhsT=pw[:, kc, :],
                                         rhs=xt[:, kc, t0:t0 + N],
                                         start=(kc == 0), stop=(kc == KC1 - 1))
                    tmp = tmpp.tile([P, 512], F32, tag="r")
                    nc.scalar.activation(tmp[:, :N], psA[:, :N], AF.Relu)
                    nc.scalar.activation(hT[:, fc, :], tmp[:, :N], AF.Square)
                # ---- B: out_chunk = hiddenT^T @ Wproj   (natural [tokens, H])
                outs = [osbp.tile([P, H], F32, tag=f"o{i}", name=f"out_sb{i}") for i in range(TC)]
                for hh in range(H // 512):
                    psBs = [psBp.tile([P, 512], F32, tag=f"B{i}", name=f"psB{i}") for i in range(TC)]
                    for kc in range(KC2):
                        pj = wpjp.tile([P, 512], F32R, tag="wpj")
                        nc.gpsimd.dma_start(out=pj[:], in_=wpj_d[hh, kc])
                        for i in range(TC):
                            nc.tensor.matmul(psBs[i][:],
                                             lhsT=hT[:, kc, i * P:(i + 1) * P],
                                             rhs=pj[:],
                                             start=(kc == 0), stop=(kc == KC2 - 1))
                    for i in range(TC):
                        nc.scalar.copy(outs[i][:, hh * 512:(hh + 1) * 512], psBs[i][:])
                for i in range(TC):
                    nc.sync.dma_start(out=out_d[t0 + i * P:t0 + (i + 1) * P, :],
                                      in_=outs[i][:])
    nc.compile()
    return nc


def _phase1_inputs(xf, gumbel_u, Wg):
    """Pure-layout host prep for phase 1 (slicing / transpose only)."""
    xT = np.ascontiguousarray(xf.T)                      # [H, T]
    wg_h = np.ascontiguousarray(
        Wg.reshape(KC1, P, E).transpose(1, 0, 2))        # [P, KC1, E]
    in_maps = []
    for c in range(8):
        sl = xT[:, c * TPC:(c + 1) * TPC]
        xTs = np.ascontiguousarray(
            sl.reshape(KC1, P, TPC).transpose(1, 0, 2))  # [P, KC1, TPC]
        u = gumbel_u[c * TPC:(c + 1) * TPC]
        u_r = np.ascontiguousarray(
            u.reshape(NT, P, E).transpose(1, 0, 2))      # [P, NT, E]
        in_maps.append({"xTs": xTs, "wg": wg_h, "u_r": u_r})
    return in_maps


def _phase2_inputs(xf, top1, Wfc, Wproj):
    """Bucket tokens by expert (host-side all-to-all dispatch: indexing only)."""
    idx_lists = [np.where(top1 == e)[0] for e in range(E)]
    n_max = max(1, max(len(ix) for ix in idx_lists))
    C = ((n_max + P - 1) // P) * P
    in_maps = []
    for e in range(E):
        ix = idx_lists[e]
        pad = np.zeros(C, dtype=np.int64)
        pad[:len(ix)] = ix
        xg = xf[pad]                                     # [C, H] gather
        xT_h = np.ascontiguousarray(
            xg.T.reshape(KC1, P, C).transpose(1, 0, 2))  # [P, KC1, C]
        wfc_h = np.ascontiguousarray(
            Wfc[e].reshape(KC1, P, KC2, P).transpose(2, 1, 0, 3))   # [KC2,P,KC1,P]
        wpj_h = np.ascontiguousarray(
            Wproj[e].reshape(KC2, P, 2, 512).transpose(2, 0, 1, 3))  # [2,KC2,P,512]
        in_maps.append({"xT": xT_h, "wfc": wfc_h, "wpj": wpj_h})
    return in_maps, idx_lists, C


def kernel(x, gumbel_u, Wg, Wfc, Wproj):
    x = np.asarray(x, dtype=np.float32)
    gumbel_u = np.asarray(gumbel_u, dtype=np.float32)
    Wg = np.asarray(Wg, dtype=np.float32)
    Wfc = np.asarray(Wfc, dtype=np.float32)
    Wproj = np.asarray(Wproj, dtype=np.float32)
    xf = x.reshape(T, H)

    # ---- phase 1: routing (on device) ----
    global _LAST_NC1, _LAST_NC2
    nc1 = _build_phase1()
    _LAST_NC1 = nc1
    r1 = run_bass_kernel_spmd(nc1, _phase1_inputs(xf, gumbel_u, Wg),
                              core_ids=list(range(8)))
    logits = np.empty((T, E), dtype=np.float32)
    top1 = np.empty(T, dtype=np.int64)
    for c in range(8):
        logits[c * TPC:(c + 1) * TPC] = r1.results[c]["logitsT"].T
        # top1 device layout [P, NT]: token t_local = tt*P + p
        top1[c * TPC:(c + 1) * TPC] = \
            r1.results[c]["top1"].astype(np.int64).T.reshape(-1)

    # ---- host dispatch (indexing only) + phase 2: expert MLPs ----
    in_maps2, idx_lists, C = _phase2_inputs(xf, top1, Wfc, Wproj)
    nc2 = _build_phase2(C)
    _LAST_NC2 = nc2
    r2 = run_bass_kernel_spmd(nc2, in_maps2, core_ids=list(range(8)))

    out = np.zeros((T, H), dtype=np.float32)
    for e in range(E):
        ix = idx_lists[e]
        if len(ix):
            out[ix] = r2.results[e]["out"][:len(ix)]

    return out.reshape(B, L, H), logits.reshape(B, L, E)


# revision 5
# speedup vs baseline: 1.1316x; 1.1316x over previous
"""MoE (soft gumbel top-2 gate, hard top-1 forward) for 8 trn2 NeuronCores.

Forward math: with TOPK=2, the reference's straight-through ``hard`` weights
are exactly one_hot(argmax) in the forward pass, so
    out[t] = relu(x[t] @ Wfc[e*])**2 @ Wproj[e*],  e* = argmax_e(logits + g)[t]
plus the prior_logits output. Strategy:
  phase 1 (data-parallel): each core computes logits^T and the gumbel-argmax
          for its 1/8 of the tokens, on device.
  host:   buckets token indices by expert (pure indexing / layout, no math),
          gathers token rows per expert, pads to a common capacity C.
  phase 2 (expert-parallel): core e runs its expert's MLP over its tokens with
          float32r matmuls (full-rate, ~1e-4 rel err).
  host:   scatters rows back and assembles the two outputs.
"""
import numpy as np

import concourse.bacc as bacc
import concourse.tile as tile
from concourse import mybir
from concourse.bass_utils import run_bass_kernel_spmd
from concourse.masks import make_identity

P = 128
B, L, H, E = 4, 2048, 1024, 8
T = B * L                 # 8192 tokens
FF = 4 * H                # 4096
TPC = T // 8              # tokens per core in phase 1
KC1 = H // P              # 8  k-chunks over H
KC2 = FF // P             # 32 k-chunks over FF
NT = TPC // P             # 8  token tiles per core in phase 1

F32 = mybir.dt.float32
F32R = mybir.dt.float32r
U32 = mybir.dt.uint32
AF = mybir.ActivationFunctionType

_LAST_NC1 = None
_LAST_NC2 = None


def _build_phase1():
    nc = bacc.Bacc("TRN2", target_bir_lowering=False, debug=False)
    xTs_d = nc.dram_tensor("xTs", [P, KC1, TPC], F32, kind="ExternalInput").ap()
    wg_d = nc.dram_tensor("wg", [P, KC1, E], F32, kind="ExternalInput").ap()
    u_d = nc.dram_tensor("u_r", [P, NT, E], F32, kind="ExternalInput").ap()
    logT_d = nc.dram_tensor("logitsT", [E, TPC], F32, kind="ExternalOutput").ap()
    top1_d = nc.dram_tensor("top1", [P, NT], U32, kind="ExternalOutput").ap()

    with tile.TileContext(nc) as tc:
        with tc.tile_pool(name="sb", bufs=1) as sb, \
             tc.tile_pool(name="w2", bufs=2) as w2, \
             tc.tile_pool(name="psL", bufs=2, space="PSUM") as psLp, \
             tc.tile_pool(name="psT", bufs=2, space="PSUM") as psTp:
            xt = sb.tile([P, KC1, TPC], F32)
            nc.sync.dma_start(out=xt[:], in_=xTs_d[:])
            wg = sb.tile([P, KC1, E], F32)
            nc.sync.dma_start(out=wg[:], in_=wg_d[:])
            us = sb.tile([P, NT, E], F32)
            nc.sync.dma_start(out=us[:], in_=u_d[:])
            ident = sb.tile([E, E], F32)
            make_identity(nc, ident[:])

            # g' = ln(-ln(clip(u)));  z = logits - g'  (tau rescale is
            # argmax-invariant, softmax/top2-renorm don't affect the forward)
            uc = sb.tile([P, NT, E], F32)
            nc.vector.tensor_scalar(uc[:], us[:], 1e-9, scalar2=None,
                                    op0=mybir.AluOpType.max)
            nc.vector.tensor_scalar(uc[:], uc[:], 1.0 - 1e-9, scalar2=None,
                                    op0=mybir.AluOpType.min)
            t1 = sb.tile([P, NT, E], F32)
            nc.scalar.activation(t1[:], uc[:], AF.Ln)
            t2 = sb.tile([P, NT, E], F32)
            nc.scalar.activation(t2[:], t1[:], AF.Ln, scale=-1.0)

            # logits^T = Wg^T @ x^T, plain fp32 for argmax fidelity
            sbL = sb.tile([E, TPC], F32)
            for st in range(TPC // 512):
                psL = psLp.tile([E, 512], F32, tag="L")
                for kc in range(KC1):
                    nc.tensor.matmul(psL[:], lhsT=wg[:, kc, :],
                                     rhs=xt[:, kc, st * 512:(st + 1) * 512],
                                     start=(kc == 0), stop=(kc == KC1 - 1))
                nc.scalar.copy(sbL[:, st * 512:(st + 1) * 512], psL[:])
            nc.sync.dma_start(out=logT_d[:], in_=sbL[:])

            top1 = sb.tile([P, NT], U32)
            for tt in range(NT):
                psT = psTp.tile([P, E], F32, tag="T")
                nc.tensor.transpose(psT[:], in_=sbL[:, tt * P:(tt + 1) * P],
                                    identity=ident[:])
                z = w2.tile([P, E], F32, tag="z")
                nc.vector.tensor_tensor(out=z[:], in0=psT[:], in1=t2[:, tt, :],
                                        op=mybir.AluOpType.subtract)
                mx = w2.tile([P, E], F32, tag="mx")
                nc.vector.max(out=mx[:], in_=z[:])
                mi = w2.tile([P, E], U32, tag="mi")
                nc.vector.max_index(mi[:], mx[:], z[:])
                nc.vector.tensor_copy(top1[:, tt:tt + 1], mi[:, 0:1])
            nc.sync.dma_start(out=top1_d[:], in_=top1[:])
    nc.compile()
    return nc


def _chunk_plan(C):
    """Token chunks, each <=512 and (when possible) >=256 so float32r
    matmuls stay at full rate."""
    chunks = []
    rem = C
    while rem > 0:
        if rem > 512:
            if rem - 512 < 256 and rem < 1024:
                # rebalance tail: e.g. 640 -> 384+256, 768 -> 512+256
                n = ((rem - 256) // 128) * 128
                n = min(512, n)
            else:
                n = 512
        else:
            n = rem
        chunks.append(n)
        rem -= n
    out, t0 = [], 0
    for n in chunks:
        out.append((t0, n))
        t0 += n
    return out


def _build_phase2(C):
    """Expert MLP over C (padded) tokens: out = relu(Wfc^T x^T)^2^T Wproj."""
    nc = bacc.Bacc("TRN2", target_bir_lowering=False, debug=False)
    xT_d = nc.dram_tensor("xT", [C * KC1 * P], F32, kind="ExternalInput").ap()
    # wfc host layout: [8 groups][P][4 fc][KC1][P]
    wfc_d = nc.dram_tensor("wfc", [8, P, 4, KC1, P], F32, kind="ExternalInput").ap()
    # wpj host layout: [2 hh][P][KC2 kc][512]
    wpj_d = nc.dram_tensor("wpj", [2, P, KC2, 512], F32, kind="ExternalInput").ap()
    out_d = nc.dram_tensor("out", [C, H], F32, kind="ExternalOutput").ap()

    chunks = _chunk_plan(C)

    with tile.TileContext(nc) as tc:
        with tc.tile_pool(name="xt", bufs=2) as xtp, \
             tc.tile_pool(name="hT", bufs=1) as hTp, \
             tc.tile_pool(name="wfc", bufs=2) as wfcp, \
             tc.tile_pool(name="wpj", bufs=2) as wpjp, \
             tc.tile_pool(name="tmp", bufs=3) as tmpp, \
             tc.tile_pool(name="osb", bufs=1) as osbp, \
             tc.tile_pool(name="psA", bufs=2, space="PSUM") as psAp, \
             tc.tile_pool(name="psB", bufs=1, space="PSUM") as psBp:
            for ci, (t0, N) in enumerate(chunks):
                TC = N // P
                xtc = xtp.tile([P, KC1, N], F32R, tag="xt", name=f"xt{ci}")
                off = t0 * KC1 * P
                nc.gpsimd.dma_start(
                    out=xtc[:],
                    in_=xT_d[off:off + N * KC1 * P].rearrange(
                        "(p k n) -> p k n", p=P, k=KC1))
                hT = hTp.tile([P, KC2, N], F32R, tag="hT")
                # ---- A: hiddenT = relu(Wfc^T @ xT_chunk)^2
                for g in range(8):
                    pw = wfcp.tile([P, 4, KC1, P], F32R, tag="wfc")
                    nc.gpsimd.dma_start(out=pw[:], in_=wfc_d[g])
                    for f in range(4):
                        fc = g * 4 + f
                        psA = psAp.tile([P, 512], F32, tag="A")
                        for kc in range(KC1):
                            nc.tensor.matmul(psA[:, :N], lhsT=pw[:, f, kc, :],
                                             rhs=xtc[:, kc, :],
                                             start=(kc == 0), stop=(kc == KC1 - 1))
                        tmp = tmpp.tile([P, 512], F32, tag="r")
                        nc.vector.tensor_scalar(tmp[:, :N], psA[:, :N], 0.0,
                                                scalar2=None,
                                                op0=mybir.AluOpType.max)
                        nc.scalar.activation(hT[:, fc, :], tmp[:, :N], AF.Square)
                # ---- B: out_chunk = hiddenT^T @ Wproj   (natural [tokens, H])
                outs = [osbp.tile([P, H], F32, tag=f"o{i}", name=f"out_sb{i}")
                        for i in range(TC)]
                for hh in range(H // 512):
                    psBs = [psBp.tile([P, 512], F32, tag=f"B{i}", name=f"psB{i}")
                            for i in range(TC)]
                    for g in range(4):
                        pj = wpjp.tile([P, 8, 512], F32R, tag="wpj")
                        nc.gpsimd.dma_start(
                            out=pj[:], in_=wpj_d[hh, :, g * 8:(g + 1) * 8, :])
                        for k in range(8):
                            kc = g * 8 + k
                            for i in range(TC):
                                nc.tensor.matmul(
                                    psBs[i][:],
                                    lhsT=hT[:, kc, i * P:(i + 1) * P],
                                    rhs=pj[:, k, :],
                                    start=(kc == 0), stop=(kc == KC2 - 1))
                    for i in range(TC):
                        nc.vector.tensor_copy(outs[i][:, hh * 512:(hh + 1) * 512],
                                              psBs[i][:])
                for i in range(TC):
                    nc.sync.dma_start(out=out_d[t0 + i * P:t0 + (i + 1) * P, :],
                                      in_=outs[i][:])
    nc.compile()
    return nc


def _phase1_inputs(xf, gumbel_u, Wg):
    """Pure-layout host prep for phase 1 (slicing / transpose only)."""
    xT = np.ascontiguousarray(xf.T)                      # [H, T]
    wg_h = np.ascontiguousarray(
        Wg.reshape(KC1, P, E).transpose(1, 0, 2))        # [P, KC1, E]
    in_maps = []
    for c in range(8):
        sl = xT[:, c * TPC:(c + 1) * TPC]
        xTs = np.ascontiguousarray(
            sl.reshape(KC1, P, TPC).transpose(1, 0, 2))  # [P, KC1, TPC]
        u = gumbel_u[c * TPC:(c + 1) * TPC]
        u_r = np.ascontiguousarray(
            u.reshape(NT, P, E).transpose(1, 0, 2))      # [P, NT, E]
        in_maps.append({"xTs": xTs, "wg": wg_h, "u_r": u_r})
    return in_maps


def _phase2_inputs(xf, top1, Wfc, Wproj):
    """Bucket tokens by expert (host-side all-to-all dispatch: indexing only)."""
    idx_lists = [np.where(top1 == e)[0] for e in range(E)]
    n_max = max(1, max(len(ix) for ix in idx_lists))
    C = ((n_max + P - 1) // P) * P
    chunks = _chunk_plan(C)
    in_maps = []
    for e in range(E):
        ix = idx_lists[e]
        pad = np.zeros(C, dtype=np.int64)
        pad[:len(ix)] = ix
        xg = xf[pad]                                     # [C, H] gather
        blocks = []
        for (t0, N) in chunks:
            blk = xg[t0:t0 + N].T.reshape(KC1, P, N).transpose(1, 0, 2)
            blocks.append(np.ascontiguousarray(blk).reshape(-1))
        xT_h = np.concatenate(blocks)                    # [C*KC1*P] chunk-major
        wfc_h = np.ascontiguousarray(
            Wfc[e].reshape(KC1, P, 8, 4, P).transpose(2, 1, 3, 0, 4))
        wpj_h = np.ascontiguousarray(
            Wproj[e].reshape(KC2, P, 2, 512).transpose(2, 1, 0, 3))
        in_maps.append({"xT": xT_h, "wfc": wfc_h, "wpj": wpj_h})
    return in_maps, idx_lists, C


def kernel(x, gumbel_u, Wg, Wfc, Wproj):
    x = np.asarray(x, dtype=np.float32)
    gumbel_u = np.asarray(gumbel_u, dtype=np.float32)
    Wg = np.asarray(Wg, dtype=np.float32)
    Wfc = np.asarray(Wfc, dtype=np.float32)
    Wproj = np.asarray(Wproj, dtype=np.float32)
    xf = x.reshape(T, H)

    # ---- phase 1: routing (on device) ----
    global _LAST_NC1, _LAST_NC2
    nc1 = _build_phase1()
    _LAST_NC1 = nc1
    r1 = run_bass_kernel_spmd(nc1, _phase1_inputs(xf, gumbel_u, Wg),
                              core_ids=list(range(8)))
    logits = np.empty((T, E), dtype=np.float32)
    top1 = np.empty(T, dtype=np.int64)
    for c in range(8):
        logits[c * TPC:(c + 1) * TPC] = r1.results[c]["logitsT"].T
        # top1 device layout [P, NT]: token t_local = tt*P + p
        top1[c * TPC:(c + 1) * TPC] = \
            r1.results[c]["top1"].astype(np.int64).T.reshape(-1)

    # ---- host dispatch (indexing only) + phase 2: expert MLPs ----
    in_maps2, idx_lists, C = _phase2_inputs(xf, top1, Wfc, Wproj)
    nc2 = _build_phase2(C)
    _LAST_NC2 = nc2
    r2 = run_bass_kernel_spmd(nc2, in_maps2, core_ids=list(range(8)))

    out = np.zeros((T, H), dtype=np.float32)
    for e in range(E):
        ix = idx_lists[e]
        if len(ix):
            out[ix] = r2.results[e]["out"][:len(ix)]

    return out.reshape(B, L, H), logits.reshape(B, L, E)


# revision 16
# speedup vs baseline: 1.3328x; 1.1778x over previous
"""MoE (soft gumbel top-2 gate, hard top-1 forward) for 8 trn2 NeuronCores.

Forward math: with TOPK=2, the reference's straight-through ``hard`` weights
are exactly one_hot(argmax) in the forward pass, so
    out[t] = relu(x[t] @ Wfc[e*])**2 @ Wproj[e*],  e* = argmax_e(logits + g)[t]
plus the prior_logits output. Strategy:
  phase 1 (data-parallel): each core computes logits^T and the gumbel-argmax
          for its 1/8 of the tokens, on device.
  host:   buckets token indices by expert (pure indexing / layout, no math),
          gathers token rows per expert, pads to a common capacity C.
  phase 2 (expert-parallel): core e runs its expert's MLP over its tokens with
          float32r matmuls (full-rate, ~1e-4 rel err).
  host:   scatters rows back and assembles the two outputs.
"""
import numpy as np

import concourse.bacc as bacc
import concourse.tile as tile
from concourse import mybir
from concourse.bass_utils import run_bass_kernel_spmd
from concourse.masks import make_identity

P = 128
B, L, H, E = 4, 2048, 1024, 8
T = B * L                 # 8192 tokens
FF = 4 * H                # 4096
TPC = T // 8              # tokens per core in phase 1
KC1 = H // P              # 8  k-chunks over H
KC2 = FF // P             # 32 k-chunks over FF
NT = TPC // P             # 8  token tiles per core in phase 1

F32 = mybir.dt.float32
F32R = mybir.dt.float32r
U32 = mybir.dt.uint32
AF = mybir.ActivationFunctionType

_LAST_NC1 = None
_LAST_NC2 = None


def _build_phase1():
    nc = bacc.Bacc("TRN2", target_bir_lowering=False, debug=False)
    xTs_d = nc.dram_tensor("xTs", [P, KC1, TPC], F32, kind="ExternalInput").ap()
    wg_d = nc.dram_tensor("wg", [P, KC1, E], F32, kind="ExternalInput").ap()
    u_d = nc.dram_tensor("u_r", [P, NT, E], F32, kind="ExternalInput").ap()
    logT_d = nc.dram_tensor("logitsT", [E, TPC], F32, kind="ExternalOutput").ap()
    top1_d = nc.dram_tensor("top1", [P, NT], U32, kind="ExternalOutput").ap()

    with tile.TileContext(nc) as tc:
        with tc.tile_pool(name="sb", bufs=1) as sb, \
             tc.tile_pool(name="w2", bufs=2) as w2, \
             tc.tile_pool(name="psL", bufs=2, space="PSUM") as psLp, \
             tc.tile_pool(name="psT", bufs=2, space="PSUM") as psTp:
            wg = sb.tile([P, KC1, E], F32)
            nc.sync.dma_start(out=wg[:], in_=wg_d[:])
            xh = []
            for st in range(2):
                x1 = sb.tile([P, KC1, 512], F32, name=f"xh{st}")
                nc.sync.dma_start(out=x1[:], in_=xTs_d[:, :, st * 512:(st + 1) * 512])
                xh.append(x1)
            us = sb.tile([P, NT, E], F32)
            nc.sync.dma_start(out=us[:], in_=u_d[:])
            ident = sb.tile([E, E], F32)
            make_identity(nc, ident[:])

            # g' = ln(-ln(clip(u)));  z = logits - g'  (tau rescale is
            # argmax-invariant, softmax/top2-renorm don't affect the forward)
            uc = sb.tile([P, NT, E], F32)
            nc.vector.tensor_scalar(uc[:], us[:], 1e-9, scalar2=None,
                                    op0=mybir.AluOpType.max)
            nc.vector.tensor_scalar(uc[:], uc[:], 1.0 - 1e-9, scalar2=None,
                                    op0=mybir.AluOpType.min)
            t1 = sb.tile([P, NT, E], F32)
            nc.scalar.activation(t1[:], uc[:], AF.Ln)
            t2 = sb.tile([P, NT, E], F32)
            nc.scalar.activation(t2[:], t1[:], AF.Ln, scale=-1.0)

            # logits^T = Wg^T @ x^T  (plain fp32 for argmax fidelity),
            # 4 kc packed into distinct 32-col groups of the PE array
            sbL = sb.tile([E, TPC], F32)
            for st in range(2):
                psL = psLp.tile([P, 512], F32, tag="L")
                for r in range(2):
                    for j in range(4):
                        kc = r * 4 + j
                        nc.tensor.matmul(psL[32 * j:32 * j + 8, :],
                                         lhsT=wg[:, kc, :],
                                         rhs=xh[st][:, kc, :],
                                         start=(r == 0), stop=(r == 1),
                                         tile_position=(0, 32 * j),
                                         skip_group_check=True)
                c0 = w2.tile([E, 512], F32, tag="c0")
                nc.scalar.copy(c0[:], psL[0:8, :])
                a1 = w2.tile([E, 512], F32, tag="a1")
                nc.vector.tensor_tensor(out=a1[:], in0=psL[32:40, :],
                                        in1=c0[:], op=mybir.AluOpType.add)
                c2 = w2.tile([E, 512], F32, tag="c2")
                nc.scalar.copy(c2[:], psL[64:72, :])
                a2 = w2.tile([E, 512], F32, tag="a2")
                nc.vector.tensor_tensor(out=a2[:], in0=psL[96:104, :],
                                        in1=c2[:], op=mybir.AluOpType.add)
                nc.vector.tensor_tensor(out=sbL[:, st * 512:(st + 1) * 512],
                                        in0=a1[:], in1=a2[:],
                                        op=mybir.AluOpType.add)
            nc.sync.dma_start(out=logT_d[:], in_=sbL[:])

            top1 = sb.tile([P, NT], U32)
            for tt in range(NT):
                psT = psTp.tile([P, E], F32, tag="T")
                nc.tensor.transpose(psT[:], in_=sbL[:, tt * P:(tt + 1) * P],
                                    identity=ident[:])
                z = w2.tile([P, E], F32, tag="z")
                nc.vector.tensor_tensor(out=z[:], in0=psT[:], in1=t2[:, tt, :],
                                        op=mybir.AluOpType.subtract)
                mx = w2.tile([P, E], F32, tag="mx")
                nc.vector.max(out=mx[:], in_=z[:])
                mi = w2.tile([P, E], U32, tag="mi")
                nc.vector.max_index(mi[:], mx[:], z[:])
                nc.vector.tensor_copy(top1[:, tt:tt + 1], mi[:, 0:1])
            nc.sync.dma_start(out=top1_d[:], in_=top1[:])
    nc.compile()
    return nc


def _chunk_plan(C):
    """Token chunks, each a multiple of 128 up to 768 (six 128-row PSUM
    accumulators in flight), avoiding tails <256 (float32r rate cliff)."""
    out, rem, t0 = [], C, 0
    while rem > 0:
        if rem <= 768:
            n = rem
        elif rem >= 768 + 256:
            n = 768
        else:
            n = rem - 256
        out.append((t0, n))
        t0 += n
        rem -= n
    return out


def _sub_plan(N):
    """<=512 sub-slices for the A-phase psum, avoiding tails <256."""
    out, rem, s0 = [], N, 0
    while rem > 0:
        if rem <= 512:
            n = rem
        elif rem >= 512 + 256:
            n = 512
        else:
            n = rem - 256
        out.append((s0, n))
        s0 += n
        rem -= n
    return out


def _build_phase2(C, reps=1):
    """Expert MLP over C (padded) tokens: out = relu(Wfc^T x^T)^2^T Wproj.

    Per chunk (<=768 tokens), per FF-half: stream Wfc panels, accumulate
    hiddenT-half in SBUF (float32r), then stream Wproj panels and accumulate
    the output in PSUM (copy for half 0, add for half 1).
    """
    nc = bacc.Bacc("TRN2", target_bir_lowering=False, debug=False)
    xT_d = nc.dram_tensor("xT", [C * KC1 * P], F32, kind="ExternalInput").ap()
    # wfc host layout: [8 groups][P][4 fc][KC1][P]   (fc = 4*group + f)
    wfc_d = nc.dram_tensor("wfc", [8, P, 4, KC1, P], F32, kind="ExternalInput").ap()
    # wpj host layout: [2 hh][P][KC2 kc][512]
    wpj_d = nc.dram_tensor("wpj", [2, P, KC2, 512], F32, kind="ExternalInput").ap()
    out_d = nc.dram_tensor("out", [C, H], F32, kind="ExternalOutput").ap()

    chunks = _chunk_plan(C)
    rep_chunks = [(t0, n) for _r in range(reps) for (t0, n) in chunks]

    with tile.TileContext(nc) as tc:
        with tc.tile_pool(name="xt", bufs=2) as xtp, \
             tc.tile_pool(name="hT", bufs=1) as hTp, \
             tc.tile_pool(name="wfc", bufs=2) as wfcp, \
             tc.tile_pool(name="wfc0", bufs=2) as wfc0p, \
             tc.tile_pool(name="wpj", bufs=2) as wpjp, \
             tc.tile_pool(name="tmp", bufs=3) as tmpp, \
             tc.tile_pool(name="osb", bufs=1) as osbp, \
             tc.tile_pool(name="psA", bufs=2, space="PSUM") as psAp, \
             tc.tile_pool(name="psB", bufs=1, space="PSUM") as psBp:
            first = True
            for ci, (t0, N) in enumerate(rep_chunks):
                TC = N // P
                subs = _sub_plan(N)
                off = t0 * KC1 * P
                full = xT_d[off:off + N * KC1 * P].rearrange(
                    "(p k n) -> p k n", p=P, k=KC1)
                if first:
                    # cold start: first fc-panel before the activations so
                    # the SWDGE queue hands PE its first operands ASAP
                    pre_pws = []
                    for f in range(4):
                        pw1 = wfc0p.tile([P, KC1, P], F32R, tag="w0",
                                         name=f"pw0_{f}")
                        nc.gpsimd.dma_start(out=pw1[:], in_=wfc_d[0, :, f])
                        pre_pws.append(pw1)
                # 4-piece activation load: first matmuls start after ~1/4
                pieces = []
                for q in range(4):
                    xq = xtp.tile([P, 2, N], F32R, tag=f"xq{q}",
                                  name=f"xq{q}_{ci}")
                    nc.gpsimd.dma_start(out=xq[:],
                                        in_=full[:, 2 * q:2 * q + 2, :])
                    pieces.append(xq)
                get_xt = lambda kc, s0, NS, _p=pieces: _p[kc // 2][:, kc % 2,
                                                               s0:s0 + NS]
                outs = [osbp.tile([P, H], F32, tag=f"o{i}", name=f"out_sb{i}")
                        for i in range(TC)]
                for half in range(2):
                    hTh = hTp.tile([P, 16, N], F32R, tag="hT", name="hTh")
                    # ---- A-half: hiddenT[half] = relu(Wfc[:,half]^T x)^2
                    for g in range(4):
                        gg = half * 4 + g
                        if first and g == 0:
                            get_pw = lambda f, kc: pre_pws[f][:, kc, :]
                        else:
                            pw = wfcp.tile([P, 4, KC1, P], F32R, tag="wfc",
                                           name="pw")
                            nc.gpsimd.dma_start(out=pw[:], in_=wfc_d[gg])
                            get_pw = lambda f, kc, _pw=pw: _pw[:, f, kc, :]
                        for f in range(4):
                            fcl = g * 4 + f
                            for (s0, NS) in subs:
                                psA = psAp.tile([P, 512], F32, tag="A")
                                for kc in range(KC1):
                                    nc.tensor.matmul(
                                        psA[:, :NS], lhsT=get_pw(f, kc),
                                        rhs=get_xt(kc, s0, NS),
                                        start=(kc == 0), stop=(kc == KC1 - 1))
                                tmp = tmpp.tile([P, 512], F32, tag="r")
                                nc.vector.tensor_scalar(
                                    tmp[:, :NS], psA[:, :NS], 0.0, scalar2=None,
                                    op0=mybir.AluOpType.max)
                                nc.scalar.activation(hTh[:, fcl, s0:s0 + NS],
                                                     tmp[:, :NS], AF.Square)
                        first = False
                    # ---- B-half: partial out += hiddenT[half]^T @ Wproj[half]
                    for hh in range(2):
                        psBs = [psBp.tile([P, 512], F32, tag=f"B{i}",
                                          name=f"psB{i}") for i in range(TC)]
                        for g in range(2):
                            pj = wpjp.tile([P, 8, 512], F32R, tag="wpj",
                                           name="pj")
                            nc.gpsimd.dma_start(
                                out=pj[:],
                                in_=wpj_d[hh, :,
                                          half * 16 + g * 8:half * 16 + g * 8 + 8,
                                          :])
                            for k in range(8):
                                kcl = g * 8 + k
                                for i in range(TC):
                                    nc.tensor.matmul(
                                        psBs[i][:],
                                        lhsT=hTh[:, kcl, i * P:(i + 1) * P],
                                        rhs=pj[:, k, :],
                                        start=(kcl == 0), stop=(kcl == 15))
                        for i in range(TC):
                            dst = outs[i][:, hh * 512:(hh + 1) * 512]
                            if half == 0:
                                nc.vector.tensor_copy(dst, psBs[i][:])
                            else:
                                nc.vector.tensor_tensor(
                                    out=dst, in0=psBs[i][:], in1=dst,
                                    op=mybir.AluOpType.add)
                for i in range(TC):
                    nc.sync.dma_start(out=out_d[t0 + i * P:t0 + (i + 1) * P, :],
                                      in_=outs[i][:])
    nc.compile()
    return nc


def _phase1_inputs(xf, gumbel_u, Wg):
    """Pure-layout host prep for phase 1 (slicing / transpose only)."""
    xT = np.ascontiguousarray(xf.T)                      # [H, T]
    wg_h = np.ascontiguousarray(
        Wg.reshape(KC1, P, E).transpose(1, 0, 2))        # [P, KC1, E]
    in_maps = []
    for c in range(8):
        sl = xT[:, c * TPC:(c + 1) * TPC]
        xTs = np.ascontiguousarray(
            sl.reshape(KC1, P, TPC).transpose(1, 0, 2))  # [P, KC1, TPC]
        u = gumbel_u[c * TPC:(c + 1) * TPC]
        u_r = np.ascontiguousarray(
            u.reshape(NT, P, E).transpose(1, 0, 2))      # [P, NT, E]
        in_maps.append({"xTs": xTs, "wg": wg_h, "u_r": u_r})
    return in_maps


def _phase2_inputs(xf, top1, Wfc, Wproj):
    """Bucket tokens by expert (host-side all-to-all dispatch: indexing only)."""
    idx_lists = [np.where(top1 == e)[0] for e in range(E)]
    n_max = max(1, max(len(ix) for ix in idx_lists))
    C = ((n_max + P - 1) // P) * P
    chunks = _chunk_plan(C)
    in_maps = []
    for e in range(E):
        ix = idx_lists[e]
        pad = np.zeros(C, dtype=np.int64)
        pad[:len(ix)] = ix
        xg = xf[pad]                                     # [C, H] gather
        blocks = []
        for (t0, N) in chunks:
            blk = xg[t0:t0 + N].T.reshape(KC1, P, N).transpose(1, 0, 2)
            blocks.append(np.ascontiguousarray(blk).reshape(-1))
        xT_h = np.concatenate(blocks)                    # [C*KC1*P] chunk-major
        wfc_h = np.ascontiguousarray(
            Wfc[e].reshape(KC1, P, 8, 4, P).transpose(2, 1, 3, 0, 4))
        wpj_h = np.ascontiguousarray(
            Wproj[e].reshape(KC2, P, 2, 512).transpose(2, 1, 0, 3))
        in_maps.append({"xT": xT_h, "wfc": wfc_h, "wpj": wpj_h})
    return in_maps, idx_lists, C


def kernel(x, gumbel_u, Wg, Wfc, Wproj):
    x = np.asarray(x, dtype=np.float32)
    gumbel_u = np.asarray(gumbel_u, dtype=np.float32)
    Wg = np.asarray(Wg, dtype=np.float32)
    Wfc = np.asarray(Wfc, dtype=np.float32)
    Wproj = np.asarray(Wproj, dtype=np.float32)
    xf = x.reshape(T, H)

    # ---- phase 1: routing (on device) ----
    global _LAST_NC1, _LAST_NC2
    nc1 = _build_phase1()
    _LAST_NC1 = nc1
    r1 = run_bass_kernel_spmd(nc1, _phase1_inputs(xf, gumbel_u, Wg),
                              core_ids=list(range(8)))
    logits = np.empty((T, E), dtype=np.float32)
    top1 = np.empty(T, dtype=np.int64)
    for c in range(8):
        logits[c * TPC:(c + 1) * TPC] = r1.results[c]["logitsT"].T
        # top1 device layout [P, NT]: token t_local = tt*P + p
        top1[c * TPC:(c + 1) * TPC] = \
            r1.results[c]["top1"].astype(np.int64).T.reshape(-1)

    # ---- host dispatch (indexing only) + phase 2: expert MLPs ----
    in_maps2, idx_lists, C = _phase2_inputs(xf, top1, Wfc, Wproj)
    nc2 = _build_phase2(C)
    _LAST_NC2 = nc2
    r2 = run_bass_kernel_spmd(nc2, in_maps2, core_ids=list(range(8)))

    out = np.zeros((T, H), dtype=np.float32)
    for e in range(E):
        ix = idx_lists[e]
        if len(ix):
            out[ix] = r2.results[e]["out"][:len(ix)]

    return out.reshape(B, L, H), logits.reshape(B, L, E)
